# revision 4
# baseline (speedup 1.0000x reference)
"""Causal self-attention Bass/Tile kernel for Trainium2, SPMD over 8 NeuronCores.

Problem (hardcoded): B=4, T=2048, C=1024, H=16 heads, D=64.
  kqv = x @ W_kqv + b_kqv ; k,q,v = split(kqv)
  att = softmax(mask(q k^T / sqrt(D)))
  y   = (att @ v) @ W_proj + b_proj

Sharding: core = b*2 + hg  (b in 0..3 batches, hg in 0..1 head-groups of 8
heads).  Each core computes its batch's q/k/v for its 8 heads, block-causal
attention, and a partial projection (its 512 channels of the contraction).
Host sums the two partials per batch and adds b_proj.

Device-side layout avoids all transposes:
  - host passes x^T (C-major), so q^T,k^T come out of the kqv matmul in
    [channel, T] layout (matmul lhsT/rhs both need the contraction on the
    partition axis), and v comes out natural [T, channel] from a second
    matmul using x^T as the stationary operand.
  - attention is computed transposed: att^T[j,i] (keys on partitions), so
    softmax denominators are per *free-dim column*.  exp() runs on the
    scalar engine straight out of PSUM with the 1/sqrt(D) scale and the
    padding-mask bias folded in.  Denominators come from a ones-matmul
    (column sums on the tensor engine), y^T = v^T @ att_sm^T accumulates in
    PSUM, and normalization is a single aligned reciprocal+multiply.
  - blocks entirely above the causal diagonal are never computed; diagonal
    128x128 blocks get a multiplicative triangular mask after exp.

All matmuls run as float32r (full fp32 storage, relaxed-precision PE mode,
1 cycle/row for free dims >= 256 -- ~4x faster than strict fp32).
"""

import numpy as np

B, T, C, H = 4, 2048, 1024, 16
D = 64            # head dim
HL = 8            # heads per core
NP = 4            # head pairs per core
P = 128
KC = C // P       # 8 contraction chunks for the kqv matmul
TI = 512          # query-tile width (matmul N)
NIT = T // TI     # 4
NJB = T // P      # 16 key blocks
NEG = -1e30

_NC = None          # cached compiled Bass program
LAST_RESULTS = None  # BassKernelResults of the most recent run (for test.py)
TRACE = False        # set True (e.g. from test.py) to capture an NTFF profile
TRACE_KW = {}


def _build_nc():
    from contextlib import ExitStack

    import concourse.mybir as mybir
    import concourse.tile as tile
    from concourse import bacc

    f32 = mybir.dt.float32
    fr = mybir.dt.float32r
    AF = mybir.ActivationFunctionType

    nc = bacc.Bacc("TRN2", target_bir_lowering=False, debug=False, num_devices=8)

    xT = nc.dram_tensor("xT", [NIT, P, KC, TI], fr, kind="ExternalInput").ap()
    wqk = nc.dram_tensor("wqk", [8, P, KC, P], fr, kind="ExternalInput").ap()
    wv = nc.dram_tensor("wv", [P, KC, HL * D], fr, kind="ExternalInput").ap()
    wpr = nc.dram_tensor("wpr", [P, NP, C], fr, kind="ExternalInput").ap()
    bqk = nc.dram_tensor("bqk", [P, 8], f32, kind="ExternalInput").ap()
    bv = nc.dram_tensor("bv", [P, HL * D], f32, kind="ExternalInput").ap()
    eb = nc.dram_tensor("eb", [P, NJB], f32, kind="ExternalInput").ap()
    tm = nc.dram_tensor("tm", [P, P], f32, kind="ExternalInput").ap()
    yout = nc.dram_tensor("yout", [T, C], f32, kind="ExternalOutput").ap()

    with tile.TileContext(nc) as tc:
        with ExitStack() as ctx:
            const = ctx.enter_context(tc.tile_pool(name="const", bufs=1))
            pers = ctx.enter_context(tc.tile_pool(name="pers", bufs=1))
            xs = ctx.enter_context(tc.tile_pool(name="xs", bufs=1))
            wqs = ctx.enter_context(tc.tile_pool(name="wqs", bufs=2))
            esp = ctx.enter_context(tc.tile_pool(name="esp", bufs=2))
            dsb = ctx.enter_context(tc.tile_pool(name="dsb", bufs=2))
            osb = ctx.enter_context(tc.tile_pool(name="osb", bufs=2))
            ypool = ctx.enter_context(tc.tile_pool(name="ypool", bufs=2))
            psmm = ctx.enter_context(tc.tile_pool(name="psmm", bufs=2, space="PSUM"))
            psatt = ctx.enter_context(tc.tile_pool(name="psatt", bufs=2, space="PSUM"))
            psyp = ctx.enter_context(tc.tile_pool(name="psyp", bufs=2, space="PSUM"))

            # long-lived tensors
            q_sb = pers.tile([P, NP, T], fr)   # q^T: partitions = 2 heads x 64 chans
            k_sb = pers.tile([P, NP, T], fr)   # k^T: same layout
            v_sb = pers.tile([P, NJB, 3 * NP, D], fr)  # per pair: [v_A | ones | v_B]
            wv_sb = pers.tile([P, KC, HL * D], fr)
            wpr_sb = pers.tile([P, NP, C], fr)
            bqk_sb = const.tile([P, 8], f32)
            bv_sb = const.tile([P, HL * D], f32)
            eb_sb = const.tile([P, NJB], f32)
            tm_sb = const.tile([P, P], f32)
            ones32 = const.tile([P, D], f32)

            nc.sync.dma_start(out=wv_sb, in_=wv)
            nc.sync.dma_start(out=wpr_sb, in_=wpr)
            nc.sync.dma_start(out=bqk_sb, in_=bqk)
            nc.sync.dma_start(out=bv_sb, in_=bv)
            nc.sync.dma_start(out=eb_sb, in_=eb)
            nc.sync.dma_start(out=tm_sb, in_=tm)
            nc.vector.memset(ones32, 1.0)
            for p_ in range(NP):
                nc.vector.tensor_copy(
                    v_sb[:, :, 3 * p_ + 1, :],
                    ones32.unsqueeze(1).to_broadcast((P, NJB, D)),
                )

            for it in range(NIT):
                q_lo = it * TI

                # ---- phase 1: q^T,k^T,v for this T-tile ----
                xt = xs.tile([P, KC, TI], fr, tag="xt")
                nc.sync.dma_start(out=xt, in_=xT[it])
                for c in range(8):
                    wt = wqs.tile([P, KC, P], fr, tag="wt")
                    nc.sync.dma_start(out=wt, in_=wqk[c])
                    ps = psmm.tile([P, TI], f32, tag="mm")
                    for k in range(KC):
                        nc.tensor.matmul(
                            ps,
                            lhsT=wt[:, k, :],
                            rhs=xt[:, k, :],
                            start=(k == 0),
                            stop=(k == KC - 1),
                        )
                    dest = k_sb if c < 4 else q_sb
                    nc.vector.tensor_scalar_add(
                        dest[:, c % 4, q_lo : q_lo + TI], ps, bqk_sb[:, c : c + 1]
                    )
                for t2 in range(4):
                    jb = it * 4 + t2
                    ps = psmm.tile([P, TI], f32, tag="mm")
                    for k in range(KC):
                        nc.tensor.matmul(
                            ps,
                            lhsT=xt[:, k, t2 * P : (t2 + 1) * P],
                            rhs=wv_sb[:, k, :],
                            start=(k == 0),
                            stop=(k == KC - 1),
                        )
                    v_jb = v_sb[:, jb].rearrange("p (pr b) d -> p pr b d", b=3)
                    ps_v = ps.rearrange("p (pr h2 d) -> p pr h2 d", pr=NP, h2=2)
                    bv_v = bv_sb.rearrange("p (pr h2 d) -> p pr h2 d", pr=NP, h2=2)
                    nc.vector.tensor_add(v_jb[:, :, 0, :], ps_v[:, :, 0, :], bv_v[:, :, 0, :])
                    nc.vector.tensor_add(v_jb[:, :, 2, :], ps_v[:, :, 1, :], bv_v[:, :, 1, :])

                # ---- phase 2: attention for this T-tile ----
                y_t = ypool.tile([P, NP, TI], fr, tag="y")
                njb = 4 * it + 4
                for pr in range(NP):
                    psA = psyp.tile([P, TI], f32, tag="py")  # [y_A ; den_A]
                    psB = psyp.tile([P, TI], f32, tag="py")  # [den_B ; y_B]
                    for jb in range(njb):
                        coff = max(0, (jb - 4 * it) * P)
                        pa = psatt.tile([P, 2, TI], f32, tag="att")
                        for h2 in range(2):
                            r0 = 64 * h2
                            nc.tensor.matmul(
                                pa[:, h2, coff:TI],
                                lhsT=k_sb[r0 : r0 + 64, pr, jb * P : (jb + 1) * P],
                                rhs=q_sb[r0 : r0 + 64, pr, q_lo + coff : q_lo + TI],
                                start=True,
                                stop=True,
                            )
                        es = esp.tile([P, 2, TI], fr, tag="es")
                        nc.scalar.activation(
                            es[:, :, coff:TI],
                            pa[:, :, coff:TI],
                            AF.Exp,
                            bias=eb_sb[:, jb : jb + 1],
                            scale=0.125,
                        )
                        if jb >= 4 * it:  # diagonal block: triangular mask
                            nc.vector.tensor_mul(
                                es[:, :, coff : coff + P],
                                es[:, :, coff : coff + P].bitcast(f32),
                                tm_sb.unsqueeze(1).to_broadcast((P, 2, P)),
                            )
                        nc.tensor.matmul(
                            psA[:, coff:TI],
                            lhsT=v_sb[:, jb, 3 * pr : 3 * pr + 2, :],
                            rhs=es[:, 0, coff:TI],
                            start=(jb == 0),
                            stop=(jb == njb - 1),
                            skip_group_check=True,
                        )
                        nc.tensor.matmul(
                            psB[:, coff:TI],
                            lhsT=v_sb[:, jb, 3 * pr + 1 : 3 * pr + 3, :],
                            rhs=es[:, 1, coff:TI],
                            start=(jb == 0),
                            stop=(jb == njb - 1),
                            skip_group_check=True,
                        )
                    rec = dsb.tile([P, TI], f32, tag="den")
                    nc.vector.reciprocal(rec[0:64, :], psA[64:128, :])
                    nc.vector.reciprocal(rec[64:128, :], psB[0:64, :])
                    nc.vector.tensor_mul(y_t[0:64, pr, :], psA[0:64, :], rec[0:64, :])
                    nc.vector.tensor_mul(y_t[64:128, pr, :], psB[64:128, :], rec[64:128, :])

                # ---- phase 3: partial projection for this T-tile ----
                for t2 in range(4):
                    st = osb.tile([P, 2, TI], f32, tag="st")
                    for n in range(2):
                        po = psmm.tile([P, TI], f32, tag="mm")
                        for kp in range(NP):
                            nc.tensor.matmul(
                                po,
                                lhsT=y_t[:, kp, t2 * P : (t2 + 1) * P],
                                rhs=wpr_sb[:, kp, n * TI : (n + 1) * TI],
                                start=(kp == 0),
                                stop=(kp == NP - 1),
                            )
                        nc.vector.tensor_copy(st[:, n, :], po)
                    row = (it * 4 + t2) * P
                    nc.sync.dma_start(
                        out=yout[row : row + P, :],
                        in_=st.rearrange("p a b -> p (a b)"),
                    )

    nc.compile()
    return nc


def _prep_core(core, x, W_kqv, b_kqv, padding_mask):
    b, hg = core // 2, core % 2
    c0 = hg * (HL * D)  # channel offset of this core's heads within each block
    xb = np.ascontiguousarray(np.asarray(x[b], dtype=np.float32))

    xT_pre = np.ascontiguousarray(
        xb.reshape(NIT, TI, KC, P).transpose(0, 3, 2, 1)
    )  # [it, p, k, t] = x[b, it*TI+t, k*P+p]

    wqk_pre = np.empty((8, P, KC, P), np.float32)
    bqk_pre = np.empty((8, P), np.float32)
    for c in range(8):
        off = (0 if c < 4 else C) + c0 + P * (c % 4)
        wqk_pre[c] = W_kqv[:, off : off + P].reshape(KC, P, P).transpose(1, 0, 2)
        bqk_pre[c] = b_kqv[off : off + P]

    wv_pre = np.ascontiguousarray(
        W_kqv[:, 2 * C + c0 : 2 * C + c0 + HL * D]
        .reshape(KC, P, HL * D)
        .transpose(1, 0, 2)
    )
    wpr_pre = np.ascontiguousarray(
        W_proj_slice(core).reshape(NP, P, C).transpose(1, 0, 2)
    )
    bv_pre = np.ascontiguousarray(
        np.broadcast_to(b_kqv[2 * C + c0 : 2 * C + c0 + HL * D], (P, HL * D))
    )
    eb_pre = np.ascontiguousarray(
        np.where(np.asarray(padding_mask[b]) != 0, 0.0, NEG)
        .astype(np.float32)
        .reshape(NJB, P)
        .T
    )
    tm_pre = np.triu(np.ones((P, P), np.float32))
    return {
        "xT": xT_pre,
        "wqk": np.ascontiguousarray(wqk_pre),
        "wv": wv_pre,
        "wpr": wpr_pre,
        "bqk": np.ascontiguousarray(bqk_pre.T),
        "bv": bv_pre,
        "eb": eb_pre,
        "tm": tm_pre,
    }


_W_PROJ = None


def W_proj_slice(core):
    hg = core % 2
    c0 = hg * (HL * D)
    return _W_PROJ[c0 : c0 + HL * D, :]


def kernel(x, W_kqv, b_kqv, W_proj, b_proj, padding_mask):
    global _NC, _W_PROJ, LAST_RESULTS
    from concourse.bass_utils import run_bass_kernel_spmd

    x = np.asarray(x, dtype=np.float32)
    W_kqv = np.asarray(W_kqv, dtype=np.float32)
    b_kqv = np.asarray(b_kqv, dtype=np.float32)
    _W_PROJ = np.asarray(W_proj, dtype=np.float32)
    b_proj = np.asarray(b_proj, dtype=np.float32)

    if _NC is None:
        _NC = _build_nc()

    in_maps = [_prep_core(c, x, W_kqv, b_kqv, padding_mask) for c in range(8)]
    res = run_bass_kernel_spmd(
        _NC, in_maps, core_ids=list(range(8)), trace=TRACE, **TRACE_KW
    )
    LAST_RESULTS = res

    out = np.empty((B, T, C), np.float32)
    for b in range(B):
        out[b] = res.results[2 * b]["yout"] + res.results[2 * b + 1]["yout"]
        out[b] += b_proj[None, :]
    return out


# revision 18
# speedup vs baseline: 1.8331x; 1.8331x over previous
"""Causal self-attention Bass/Tile kernel for Trainium2, SPMD over 8 NeuronCores.

Problem (hardcoded): B=4, T=2048, C=1024, H=16 heads, D=64.
  kqv = x @ W_kqv + b_kqv ; k,q,v = split(kqv)
  att = softmax(mask(q k^T / sqrt(D)))
  y   = (att @ v) @ W_proj + b_proj

Sharding: core = b*2 + hg  (b in 0..3 batches, hg in 0..1 head-groups of 8
heads).  Each core computes its batch's q/k/v for its 8 heads, block-causal
attention, and a partial projection (its 512 channels of the contraction).
Host sums the two partials per batch and adds b_proj.

Device-side layout avoids all transposes:
  - host passes x^T (C-major), so q^T,k^T come out of the kqv matmul in
    [channel, T] layout (matmul lhsT/rhs both need the contraction on the
    partition axis), and v comes out natural [T, channel] from a second
    matmul using x^T as the stationary operand.
  - attention is computed transposed: att^T[j,i] (keys on partitions), so
    softmax denominators are per *free-dim column*.  exp() runs on the
    scalar engine straight out of PSUM with the 1/sqrt(D) scale and the
    padding-mask bias folded in.  Denominators come from a ones-matmul
    (column sums on the tensor engine), y^T = v^T @ att_sm^T accumulates in
    PSUM, and normalization is a single aligned reciprocal+multiply.
  - blocks entirely above the causal diagonal are never computed; diagonal
    128x128 blocks get a multiplicative triangular mask after exp.

All matmuls run as float32r (full fp32 storage, relaxed-precision PE mode,
1 cycle/row for free dims >= 256 -- ~4x faster than strict fp32).
"""

import numpy as np

B, T, C, H = 4, 2048, 1024, 16
D = 64            # head dim
HL = 8            # heads per core
NP = 4            # head pairs per core
P = 128
KC = C // P       # 8 contraction chunks for the kqv matmul
TI = 512          # query-tile width (matmul N)
NIT = T // TI     # 4
NJB = T // P      # 16 key blocks
NEG = -1e30

COMPUTE = "bf16"    # "bf16" (2x matmul throughput) or "fp32r" (max accuracy)
_NC = {}            # compute-dtype -> cached compiled Bass program
LAST_RESULTS = None  # BassKernelResults of the most recent run (for test.py)
TRACE = False        # set True (e.g. from test.py) to capture an NTFF profile
TRACE_KW = {}


def _build_nc(compute=None):
    from contextlib import ExitStack

    import concourse.mybir as mybir
    import concourse.tile as tile
    from concourse import bacc

    f32 = mybir.dt.float32
    is_bf = (compute or COMPUTE) != "fp32r"
    fr = mybir.dt.bfloat16 if is_bf else mybir.dt.float32r
    AF = mybir.ActivationFunctionType

    nc = bacc.Bacc("TRN2", target_bir_lowering=False, debug=False, num_devices=8)

    xT = nc.dram_tensor("xT", [NIT, P, KC, TI], fr, kind="ExternalInput").ap()
    wqk = nc.dram_tensor("wqk", [8, P, KC, P], fr, kind="ExternalInput").ap()
    wv = nc.dram_tensor("wv", [P, KC, HL * D], fr, kind="ExternalInput").ap()
    wpr = nc.dram_tensor("wpr", [P, NP, C], fr, kind="ExternalInput").ap()
    bqk = nc.dram_tensor("bqk", [P, 8], f32, kind="ExternalInput").ap()
    bv = nc.dram_tensor("bv", [P, HL * D], f32, kind="ExternalInput").ap()
    eb = nc.dram_tensor("eb", [P, NJB], f32, kind="ExternalInput").ap()
    tm = nc.dram_tensor("tm", [P, P], fr, kind="ExternalInput").ap()
    yout = nc.dram_tensor("yout", [T, C], f32, kind="ExternalOutput").ap()

    with tile.TileContext(nc) as tc:
        with ExitStack() as ctx:
            const = ctx.enter_context(tc.tile_pool(name="const", bufs=1))
            pers = ctx.enter_context(tc.tile_pool(name="pers", bufs=1))
            xs = ctx.enter_context(tc.tile_pool(name="xs", bufs=2 if is_bf else 1))
            wqs = ctx.enter_context(tc.tile_pool(name="wqs", bufs=3 if is_bf else 2))
            esp = ctx.enter_context(tc.tile_pool(name="esp", bufs=3 if is_bf else 2))
            dsb = ctx.enter_context(tc.tile_pool(name="dsb", bufs=2))
            osb = ctx.enter_context(tc.tile_pool(name="osb", bufs=2))
            ypool = ctx.enter_context(tc.tile_pool(name="ypool", bufs=4 if is_bf else 2))
            psmm = ctx.enter_context(tc.tile_pool(name="psmm", bufs=2, space="PSUM"))
            psyp = ctx.enter_context(tc.tile_pool(name="psyp", bufs=2, space="PSUM"))
            psatt = ctx.enter_context(tc.tile_pool(name="psatt", bufs=2, space="PSUM"))

            # long-lived tensors
            q_sb = pers.tile([P, NP, T], fr)   # q^T: partitions = 2 heads x 64 chans
            k_sb = pers.tile([P, NP, T], fr)   # k^T: same layout
            v_sb = pers.tile([P, NJB, 3 * NP, D], fr)  # per pair: [v_A | ones | v_B]
            wv_sb = pers.tile([P, KC, HL * D], fr)
            wpr_sb = pers.tile([P, NP, C], fr)
            bqk_sb = const.tile([P, 8], f32)
            bv_sb = const.tile([P, HL * D], f32)
            eb_sb = const.tile([P, NJB], f32)
            tm_sb = const.tile([P, P], fr)
            ones32 = const.tile([P, D], f32)

            from collections import deque

            def load_consts():
                nc.sync.dma_start(out=bqk_sb, in_=bqk)
                nc.sync.dma_start(out=wv_sb, in_=wv)
                nc.sync.dma_start(out=bv_sb, in_=bv)
                nc.sync.dma_start(out=eb_sb, in_=eb)
                nc.sync.dma_start(out=tm_sb, in_=tm)
                nc.sync.dma_start(out=wpr_sb, in_=wpr)
                nc.vector.memset(ones32, 1.0)
                for p_ in range(NP):
                    nc.vector.tensor_copy(
                        v_sb[:, :, 3 * p_ + 1, :],
                        ones32.unsqueeze(1).to_broadcast((P, NJB, D)),
                    )


            def phase1_thunks(it):
                q_lo = it * TI
                xt = xs.tile([P, KC, TI], fr, tag="xt")
                nc.sync.dma_start(out=xt, in_=xT[it])
                thunks = []

                def qk_chunk(c, xt=xt, q_lo=q_lo):
                    wt = wqs.tile([P, KC, P], fr, tag="wt")
                    nc.sync.dma_start(out=wt, in_=wqk[c])
                    ps = psmm.tile([P, TI], f32, tag="mm")
                    for k in range(KC):
                        nc.tensor.matmul(
                            ps,
                            lhsT=wt[:, k, :],
                            rhs=xt[:, k, :],
                            start=(k == 0),
                            stop=(k == KC - 1),
                        )
                    dest = k_sb if c < 4 else q_sb
                    nc.vector.tensor_scalar_add(
                        dest[:, c % 4, q_lo : q_lo + TI], ps, bqk_sb[:, c : c + 1]
                    )

                def v_chunk(t2, xt=xt, it=it):
                    jb = it * 4 + t2
                    ps = psmm.tile([P, TI], f32, tag="mm")
                    for k in range(KC):
                        nc.tensor.matmul(
                            ps,
                            lhsT=xt[:, k, t2 * P : (t2 + 1) * P],
                            rhs=wv_sb[:, k, :],
                            start=(k == 0),
                            stop=(k == KC - 1),
                        )
                    v_jb = v_sb[:, jb].rearrange("p (pr b) d -> p pr b d", b=3)
                    ps_v = ps.rearrange("p (pr h2 d) -> p pr h2 d", pr=NP, h2=2)
                    bv_v = bv_sb.rearrange("p (pr h2 d) -> p pr h2 d", pr=NP, h2=2)
                    nc.vector.tensor_add(v_jb[:, :, 0, :], ps_v[:, :, 0, :], bv_v[:, :, 0, :])
                    nc.vector.tensor_add(v_jb[:, :, 2, :], ps_v[:, :, 1, :], bv_v[:, :, 1, :])

                for c in range(8):
                    thunks.append(lambda c=c: qk_chunk(c))
                for t2 in range(4):
                    thunks.append(lambda t2=t2: v_chunk(t2))
                return thunks

            def proj_thunks(it, y_t):
                def t2_chunk(t2, y_t=y_t, it=it):
                    st = osb.tile([P, 2, TI], f32, tag="st")
                    for n in range(2):
                        po = psmm.tile([P, TI], f32, tag="mm")
                        for kp in range(NP):
                            nc.tensor.matmul(
                                po,
                                lhsT=y_t[:, kp, t2 * P : (t2 + 1) * P],
                                rhs=wpr_sb[:, kp, n * TI : (n + 1) * TI],
                                start=(kp == 0),
                                stop=(kp == NP - 1),
                            )
                        nc.vector.tensor_copy(st[:, n, :], po)
                    row = (it * 4 + t2) * P
                    nc.sync.dma_start(
                        out=yout[row : row + P, :],
                        in_=st.rearrange("p a b -> p (a b)"),
                    )

                return [lambda t2=t2: t2_chunk(t2) for t2 in range(4)]

            def attention(it, y_t, pop_hook):
                q_lo = it * TI
                njb = 4 * it + 4
                cnt = 0
                for pr in range(NP):
                    psA = psyp.tile([P, TI], f32, tag="py")  # [y_A ; den_A]
                    psB = psyp.tile([P, TI], f32, tag="py")  # [den_B ; y_B]
                    for jb in range(njb):
                        coff = max(0, (jb - 4 * it) * P)
                        pa = psatt.tile([P, 2, TI], f32, tag="att")
                        for h2 in range(2):
                            r0 = 64 * h2
                            nc.tensor.matmul(
                                pa[:, h2, coff:TI],
                                lhsT=k_sb[r0 : r0 + 64, pr, jb * P : (jb + 1) * P],
                                rhs=q_sb[r0 : r0 + 64, pr, q_lo + coff : q_lo + TI],
                                start=True,
                                stop=True,
                            )
                        es = esp.tile([P, 2, TI], fr, tag="es")
                        nc.scalar.activation(
                            es[:, :, coff:TI],
                            pa[:, :, coff:TI],
                            AF.Exp,
                            bias=eb_sb[:, jb : jb + 1],
                            scale=0.125,
                        )
                        if jb >= 4 * it:  # diagonal block: triangular mask
                            nc.vector.tensor_mul(
                                es[:, :, coff : coff + P],
                                es[:, :, coff : coff + P].bitcast(
                                    f32 if fr == mybir.dt.float32r else fr
                                ),
                                tm_sb.unsqueeze(1).to_broadcast((P, 2, P)),
                            )
                        nc.tensor.matmul(
                            psA[:, coff:TI],
                            lhsT=v_sb[:, jb, 3 * pr : 3 * pr + 2, :],
                            rhs=es[:, 0, coff:TI],
                            start=(jb == 0),
                            stop=(jb == njb - 1),
                            skip_group_check=True,
                        )
                        nc.tensor.matmul(
                            psB[:, coff:TI],
                            lhsT=v_sb[:, jb, 3 * pr + 1 : 3 * pr + 3, :],
                            rhs=es[:, 1, coff:TI],
                            start=(jb == 0),
                            stop=(jb == njb - 1),
                            skip_group_check=True,
                        )
                        cnt += 1
                        pop_hook(cnt)
                    pop_hook(0)
                    den = dsb.tile([P, TI], f32, tag="nrm")
                    nc.vector.tensor_copy(den[0:64, :], psA[64:128, :])
                    nc.vector.tensor_copy(den[64:128, :], psB[0:64, :])
                    rec = dsb.tile([P, TI], f32, tag="nrm")
                    nc.vector.reciprocal_approx_fast(rec, den)
                    nc.vector.tensor_mul(y_t[0:64, pr, :], psA[0:64, :], rec[0:64, :])
                    nc.vector.tensor_mul(y_t[64:128, pr, :], psB[64:128, :], rec[64:128, :])

            if is_bf:
                fillA = deque()  # phase1: must finish before its attention tile
                fillB = deque()  # deferred projection: pure filler

                def pop_fill(_cnt=None):
                    if fillA:
                        fillA.popleft()()
                    elif fillB:
                        fillB.popleft()()

                def pop_hook(cnt):
                    if (fillA or fillB) and cnt % 3 == 0:
                        pop_fill()

                fillA.extend(phase1_thunks(0))
                load_consts()
                for it in range(NIT):
                    while fillA:
                        fillA.popleft()()
                    if it + 1 < NIT:
                        fillA.extend(phase1_thunks(it + 1))
                    y_t = ypool.tile([P, NP, TI], fr, tag="y")
                    attention(it, y_t, pop_hook)
                    fillB.extend(proj_thunks(it, y_t))
                while fillA or fillB:
                    pop_fill()
            else:
                load_consts()
                for it in range(NIT):
                    for th in phase1_thunks(it):
                        th()
                    y_t = ypool.tile([P, NP, TI], fr, tag="y")
                    attention(it, y_t, lambda cnt: None)
                    for th in proj_thunks(it, y_t):
                        th()

    nc.compile()
    return nc


def _np_compute_dtype():
    if COMPUTE == "fp32r":
        return np.float32
    import ml_dtypes

    return ml_dtypes.bfloat16


def _prep_core(core, x, W_kqv, b_kqv, padding_mask):
    cdt = _np_compute_dtype()
    b, hg = core // 2, core % 2
    c0 = hg * (HL * D)  # channel offset of this core's heads within each block
    xb = np.ascontiguousarray(np.asarray(x[b], dtype=np.float32))

    xT_pre = np.ascontiguousarray(
        xb.reshape(NIT, TI, KC, P).transpose(0, 3, 2, 1)
    ).astype(cdt)  # [it, p, k, t] = x[b, it*TI+t, k*P+p]

    wqk_pre = np.empty((8, P, KC, P), np.float32)
    bqk_pre = np.empty((8, P), np.float32)
    for c in range(8):
        off = (0 if c < 4 else C) + c0 + P * (c % 4)
        wqk_pre[c] = W_kqv[:, off : off + P].reshape(KC, P, P).transpose(1, 0, 2)
        bqk_pre[c] = b_kqv[off : off + P]

    wv_pre = np.ascontiguousarray(
        W_kqv[:, 2 * C + c0 : 2 * C + c0 + HL * D]
        .reshape(KC, P, HL * D)
        .transpose(1, 0, 2)
    )
    wpr_pre = np.ascontiguousarray(
        W_proj_slice(core).reshape(NP, P, C).transpose(1, 0, 2)
    )
    bv_pre = np.ascontiguousarray(
        np.broadcast_to(b_kqv[2 * C + c0 : 2 * C + c0 + HL * D], (P, HL * D))
    )
    eb_pre = np.ascontiguousarray(
        np.where(np.asarray(padding_mask[b]) != 0, 0.0, NEG)
        .astype(np.float32)
        .reshape(NJB, P)
        .T
    )
    tm_pre = np.triu(np.ones((P, P), np.float32)).astype(cdt)
    return {
        "xT": xT_pre,
        "wqk": np.ascontiguousarray(wqk_pre).astype(cdt),
        "wv": wv_pre.astype(cdt),
        "wpr": wpr_pre.astype(cdt),
        "bqk": np.ascontiguousarray(bqk_pre.T),
        "bv": bv_pre,
        "eb": eb_pre,
        "tm": tm_pre,
    }


_W_PROJ = None


def W_proj_slice(core):
    hg = core % 2
    c0 = hg * (HL * D)
    return _W_PROJ[c0 : c0 + HL * D, :]


def kernel(x, W_kqv, b_kqv, W_proj, b_proj, padding_mask):
    global _NC, _W_PROJ, LAST_RESULTS
    from concourse.bass_utils import run_bass_kernel_spmd

    x = np.asarray(x, dtype=np.float32)
    W_kqv = np.asarray(W_kqv, dtype=np.float32)
    b_kqv = np.asarray(b_kqv, dtype=np.float32)
    _W_PROJ = np.asarray(W_proj, dtype=np.float32)
    b_proj = np.asarray(b_proj, dtype=np.float32)

    if COMPUTE not in _NC:
        _NC[COMPUTE] = _build_nc(COMPUTE)

    in_maps = [_prep_core(c, x, W_kqv, b_kqv, padding_mask) for c in range(8)]
    res = run_bass_kernel_spmd(
        _NC[COMPUTE], in_maps, core_ids=list(range(8)), trace=TRACE, **TRACE_KW
    )
    LAST_RESULTS = res

    out = np.empty((B, T, C), np.float32)
    for b in range(B):
        out[b] = res.results[2 * b]["yout"] + res.results[2 * b + 1]["yout"]
        out[b] += b_proj[None, :]
    return out


# revision 20
# speedup vs baseline: 1.8596x; 1.0144x over previous
"""Causal self-attention Bass/Tile kernel for Trainium2, SPMD over 8 NeuronCores.

Problem (hardcoded): B=4, T=2048, C=1024, H=16 heads, D=64.
  kqv = x @ W_kqv + b_kqv ; k,q,v = split(kqv)
  att = softmax(mask(q k^T / sqrt(D)))
  y   = (att @ v) @ W_proj + b_proj

Sharding: core = b*2 + hg  (b in 0..3 batches, hg in 0..1 head-groups of 8
heads).  Each core computes its batch's q/k/v for its 8 heads, block-causal
attention, and a partial projection (its 512 channels of the contraction).
Host sums the two partials per batch and adds b_proj.

Device-side layout avoids all transposes:
  - host passes x^T (C-major), so q^T,k^T come out of the kqv matmul in
    [channel, T] layout (matmul lhsT/rhs both need the contraction on the
    partition axis), and v comes out natural [T, channel] from a second
    matmul using x^T as the stationary operand.
  - attention is computed transposed: att^T[j,i] (keys on partitions), so
    softmax denominators are per *free-dim column*.  exp() runs on the
    scalar engine straight out of PSUM with the 1/sqrt(D) scale and the
    padding-mask bias folded in.  Denominators come from a ones-matmul
    (column sums on the tensor engine), y^T = v^T @ att_sm^T accumulates in
    PSUM, and normalization is a single aligned reciprocal+multiply.
  - blocks entirely above the causal diagonal are never computed; diagonal
    128x128 blocks get a multiplicative triangular mask after exp.

All matmuls run as float32r (full fp32 storage, relaxed-precision PE mode,
1 cycle/row for free dims >= 256 -- ~4x faster than strict fp32).
"""

import numpy as np

B, T, C, H = 4, 2048, 1024, 16
D = 64            # head dim
HL = 8            # heads per core
NP = 4            # head pairs per core
P = 128
KC = C // P       # 8 contraction chunks for the kqv matmul
TI = 512          # query-tile width (matmul N)
NIT = T // TI     # 4
NJB = T // P      # 16 key blocks
NEG = -1e30

COMPUTE = "bf16"    # "bf16" (2x matmul throughput) or "fp32r" (max accuracy)
_NC = {}            # compute-dtype -> cached compiled Bass program
LAST_RESULTS = None  # BassKernelResults of the most recent run (for test.py)
TRACE = False        # set True (e.g. from test.py) to capture an NTFF profile
TRACE_KW = {}


def _build_nc(compute=None):
    from contextlib import ExitStack

    import concourse.mybir as mybir
    import concourse.tile as tile
    from concourse import bacc

    f32 = mybir.dt.float32
    is_bf = (compute or COMPUTE) != "fp32r"
    fr = mybir.dt.bfloat16 if is_bf else mybir.dt.float32r
    AF = mybir.ActivationFunctionType

    nc = bacc.Bacc("TRN2", target_bir_lowering=False, debug=False, num_devices=8)

    xT = nc.dram_tensor("xT", [NIT, P, KC, TI], fr, kind="ExternalInput").ap()
    wqk = nc.dram_tensor("wqk", [8, P, KC, P], fr, kind="ExternalInput").ap()
    wv = nc.dram_tensor("wv", [P, KC, HL * D], fr, kind="ExternalInput").ap()
    wpr = nc.dram_tensor("wpr", [P, NP, C], fr, kind="ExternalInput").ap()
    bqk = nc.dram_tensor("bqk", [P, 8], f32, kind="ExternalInput").ap()
    bv = nc.dram_tensor("bv", [P, HL * D], f32, kind="ExternalInput").ap()
    eb = nc.dram_tensor("eb", [P, NJB], f32, kind="ExternalInput").ap()
    tm = nc.dram_tensor("tm", [P, P], fr, kind="ExternalInput").ap()
    yout = nc.dram_tensor("yout", [T, C], f32, kind="ExternalOutput").ap()

    with tile.TileContext(nc) as tc:
        with ExitStack() as ctx:
            const = ctx.enter_context(tc.tile_pool(name="const", bufs=1))
            pers = ctx.enter_context(tc.tile_pool(name="pers", bufs=1))
            xs = ctx.enter_context(tc.tile_pool(name="xs", bufs=2 if is_bf else 1))
            wqs = ctx.enter_context(tc.tile_pool(name="wqs", bufs=3 if is_bf else 2))
            esp = ctx.enter_context(tc.tile_pool(name="esp", bufs=4 if is_bf else 2))
            dsb = ctx.enter_context(tc.tile_pool(name="dsb", bufs=4 if is_bf else 2))
            osb = ctx.enter_context(tc.tile_pool(name="osb", bufs=2))
            ypool = ctx.enter_context(tc.tile_pool(name="ypool", bufs=4 if is_bf else 2))
            psmm = ctx.enter_context(tc.tile_pool(name="psmm", bufs=2, space="PSUM"))
            psyp = ctx.enter_context(tc.tile_pool(name="psyp", bufs=2, space="PSUM"))
            psatt = ctx.enter_context(tc.tile_pool(name="psatt", bufs=2, space="PSUM"))

            # long-lived tensors
            q_sb = pers.tile([P, NP, T], fr)   # q^T: partitions = 2 heads x 64 chans
            k_sb = pers.tile([P, NP, T], fr)   # k^T: same layout
            v_sb = pers.tile([P, NJB, 3 * NP, D], fr)  # per pair: [v_A | ones | v_B]
            wv_sb = pers.tile([P, KC, HL * D], fr)
            wpr_sb = pers.tile([P, NP, C], fr)
            bqk_sb = const.tile([P, 8], f32)
            bv_sb = const.tile([P, HL * D], f32)
            eb_sb = const.tile([P, NJB], f32)
            tm_sb = const.tile([P, P], fr)
            ones32 = const.tile([P, D], f32)

            nc.sync.dma_start(out=wv_sb, in_=wv)
            nc.sync.dma_start(out=wpr_sb, in_=wpr)
            nc.sync.dma_start(out=bqk_sb, in_=bqk)
            nc.sync.dma_start(out=bv_sb, in_=bv)
            nc.sync.dma_start(out=eb_sb, in_=eb)
            nc.sync.dma_start(out=tm_sb, in_=tm)
            nc.vector.memset(ones32, 1.0)
            for p_ in range(NP):
                nc.vector.tensor_copy(
                    v_sb[:, :, 3 * p_ + 1, :],
                    ones32.unsqueeze(1).to_broadcast((P, NJB, D)),
                )

            from collections import deque

            def phase1_thunks(it):
                q_lo = it * TI
                xt = xs.tile([P, KC, TI], fr, tag="xt")
                nc.sync.dma_start(out=xt, in_=xT[it])
                thunks = []

                def qk_chunk(c, xt=xt, q_lo=q_lo):
                    wt = wqs.tile([P, KC, P], fr, tag="wt")
                    nc.sync.dma_start(out=wt, in_=wqk[c])
                    ps = psmm.tile([P, TI], f32, tag="mm")
                    for k in range(KC):
                        nc.tensor.matmul(
                            ps,
                            lhsT=wt[:, k, :],
                            rhs=xt[:, k, :],
                            start=(k == 0),
                            stop=(k == KC - 1),
                        )
                    dest = k_sb if c < 4 else q_sb
                    nc.vector.tensor_scalar_add(
                        dest[:, c % 4, q_lo : q_lo + TI], ps, bqk_sb[:, c : c + 1]
                    )

                def v_chunk(t2, xt=xt, it=it):
                    jb = it * 4 + t2
                    ps = psmm.tile([P, TI], f32, tag="mm")
                    for k in range(KC):
                        nc.tensor.matmul(
                            ps,
                            lhsT=xt[:, k, t2 * P : (t2 + 1) * P],
                            rhs=wv_sb[:, k, :],
                            start=(k == 0),
                            stop=(k == KC - 1),
                        )
                    v_jb = v_sb[:, jb].rearrange("p (pr b) d -> p pr b d", b=3)
                    ps_v = ps.rearrange("p (pr h2 d) -> p pr h2 d", pr=NP, h2=2)
                    bv_v = bv_sb.rearrange("p (pr h2 d) -> p pr h2 d", pr=NP, h2=2)
                    nc.vector.tensor_add(v_jb[:, :, 0, :], ps_v[:, :, 0, :], bv_v[:, :, 0, :])
                    nc.vector.tensor_add(v_jb[:, :, 2, :], ps_v[:, :, 1, :], bv_v[:, :, 1, :])

                for c in range(8):
                    thunks.append(lambda c=c: qk_chunk(c))
                for t2 in range(4):
                    thunks.append(lambda t2=t2: v_chunk(t2))
                return thunks

            def proj_thunks(it, y_t):
                def t2_chunk(t2, y_t=y_t, it=it):
                    st = osb.tile([P, 2, TI], f32, tag="st")
                    for n in range(2):
                        po = psmm.tile([P, TI], f32, tag="mm")
                        for kp in range(NP):
                            nc.tensor.matmul(
                                po,
                                lhsT=y_t[:, kp, t2 * P : (t2 + 1) * P],
                                rhs=wpr_sb[:, kp, n * TI : (n + 1) * TI],
                                start=(kp == 0),
                                stop=(kp == NP - 1),
                            )
                        nc.vector.tensor_copy(st[:, n, :], po)
                    row = (it * 4 + t2) * P
                    nc.sync.dma_start(
                        out=yout[row : row + P, :],
                        in_=st.rearrange("p a b -> p (a b)"),
                    )

                return [lambda t2=t2: t2_chunk(t2) for t2 in range(4)]

            def attention(it, y_t, pop_hook):
                q_lo = it * TI
                njb = 4 * it + 4
                cnt = 0
                for pr in range(NP):
                    psA = psyp.tile([P, TI], f32, tag="py")  # [y_A ; den_A]
                    psB = psyp.tile([P, TI], f32, tag="py")  # [den_B ; y_B]
                    for jb in range(njb):
                        coff = max(0, (jb - 4 * it) * P)
                        pa = psatt.tile([P, 2, TI], f32, tag="att")
                        for h2 in range(2):
                            r0 = 64 * h2
                            nc.tensor.matmul(
                                pa[:, h2, coff:TI],
                                lhsT=k_sb[r0 : r0 + 64, pr, jb * P : (jb + 1) * P],
                                rhs=q_sb[r0 : r0 + 64, pr, q_lo + coff : q_lo + TI],
                                start=True,
                                stop=True,
                            )
                        es = esp.tile([P, 2, TI], fr, tag="es")
                        nc.scalar.activation(
                            es[:, :, coff:TI],
                            pa[:, :, coff:TI],
                            AF.Exp,
                            bias=eb_sb[:, jb : jb + 1],
                            scale=0.125,
                        )
                        if jb >= 4 * it:  # diagonal block: triangular mask
                            nc.vector.tensor_mul(
                                es[:, :, coff : coff + P],
                                es[:, :, coff : coff + P].bitcast(
                                    f32 if fr == mybir.dt.float32r else fr
                                ),
                                tm_sb.unsqueeze(1).to_broadcast((P, 2, P)),
                            )
                        nc.tensor.matmul(
                            psA[:, coff:TI],
                            lhsT=v_sb[:, jb, 3 * pr : 3 * pr + 2, :],
                            rhs=es[:, 0, coff:TI],
                            start=(jb == 0),
                            stop=(jb == njb - 1),
                            skip_group_check=True,
                        )
                        nc.tensor.matmul(
                            psB[:, coff:TI],
                            lhsT=v_sb[:, jb, 3 * pr + 1 : 3 * pr + 3, :],
                            rhs=es[:, 1, coff:TI],
                            start=(jb == 0),
                            stop=(jb == njb - 1),
                            skip_group_check=True,
                        )
                        cnt += 1
                        pop_hook(cnt)
                    pop_hook(0)
                    den = dsb.tile([P, TI], f32, tag="nrm")
                    nc.vector.tensor_copy(den[0:64, :], psA[64:128, :])
                    nc.vector.tensor_copy(den[64:128, :], psB[0:64, :])
                    rec = dsb.tile([P, TI], f32, tag="nrm")
                    nc.vector.reciprocal_approx_fast(rec, den)
                    nc.vector.tensor_mul(y_t[0:64, pr, :], psA[0:64, :], rec[0:64, :])
                    nc.vector.tensor_mul(y_t[64:128, pr, :], psB[64:128, :], rec[64:128, :])

            if is_bf:
                fillA = deque()  # phase1: must finish before its attention tile
                fillB = deque()  # deferred projection: pure filler

                def pop_fill(_cnt=None):
                    if fillA:
                        fillA.popleft()()
                    elif fillB:
                        fillB.popleft()()

                def pop_hook(cnt):
                    if (fillA or fillB) and cnt % 3 == 0:
                        pop_fill()

                fillA.extend(phase1_thunks(0))
                for it in range(NIT):
                    while fillA:
                        fillA.popleft()()
                    if it + 1 < NIT:
                        fillA.extend(phase1_thunks(it + 1))
                    y_t = ypool.tile([P, NP, TI], fr, tag="y")
                    attention(it, y_t, pop_hook)
                    fillB.extend(proj_thunks(it, y_t))
                while fillA or fillB:
                    pop_fill()
            else:
                for it in range(NIT):
                    for th in phase1_thunks(it):
                        th()
                    y_t = ypool.tile([P, NP, TI], fr, tag="y")
                    attention(it, y_t, lambda cnt: None)
                    for th in proj_thunks(it, y_t):
                        th()

    nc.compile()
    return nc


def _np_compute_dtype():
    if COMPUTE == "fp32r":
        return np.float32
    import ml_dtypes

    return ml_dtypes.bfloat16


def _prep_core(core, x, W_kqv, b_kqv, padding_mask):
    cdt = _np_compute_dtype()
    b, hg = core // 2, core % 2
    c0 = hg * (HL * D)  # channel offset of this core's heads within each block
    xb = np.ascontiguousarray(np.asarray(x[b], dtype=np.float32))

    xT_pre = np.ascontiguousarray(
        xb.reshape(NIT, TI, KC, P).transpose(0, 3, 2, 1)
    ).astype(cdt)  # [it, p, k, t] = x[b, it*TI+t, k*P+p]

    wqk_pre = np.empty((8, P, KC, P), np.float32)
    bqk_pre = np.empty((8, P), np.float32)
    for c in range(8):
        off = (0 if c < 4 else C) + c0 + P * (c % 4)
        wqk_pre[c] = W_kqv[:, off : off + P].reshape(KC, P, P).transpose(1, 0, 2)
        bqk_pre[c] = b_kqv[off : off + P]

    wv_pre = np.ascontiguousarray(
        W_kqv[:, 2 * C + c0 : 2 * C + c0 + HL * D]
        .reshape(KC, P, HL * D)
        .transpose(1, 0, 2)
    )
    wpr_pre = np.ascontiguousarray(
        W_proj_slice(core).reshape(NP, P, C).transpose(1, 0, 2)
    )
    bv_pre = np.ascontiguousarray(
        np.broadcast_to(b_kqv[2 * C + c0 : 2 * C + c0 + HL * D], (P, HL * D))
    )
    eb_pre = np.ascontiguousarray(
        np.where(np.asarray(padding_mask[b]) != 0, 0.0, NEG)
        .astype(np.float32)
        .reshape(NJB, P)
        .T
    )
    tm_pre = np.triu(np.ones((P, P), np.float32)).astype(cdt)
    return {
        "xT": xT_pre,
        "wqk": np.ascontiguousarray(wqk_pre).astype(cdt),
        "wv": wv_pre.astype(cdt),
        "wpr": wpr_pre.astype(cdt),
        "bqk": np.ascontiguousarray(bqk_pre.T),
        "bv": bv_pre,
        "eb": eb_pre,
        "tm": tm_pre,
    }


_W_PROJ = None


def W_proj_slice(core):
    hg = core % 2
    c0 = hg * (HL * D)
    return _W_PROJ[c0 : c0 + HL * D, :]


def kernel(x, W_kqv, b_kqv, W_proj, b_proj, padding_mask):
    global _NC, _W_PROJ, LAST_RESULTS
    from concourse.bass_utils import run_bass_kernel_spmd

    x = np.asarray(x, dtype=np.float32)
    W_kqv = np.asarray(W_kqv, dtype=np.float32)
    b_kqv = np.asarray(b_kqv, dtype=np.float32)
    _W_PROJ = np.asarray(W_proj, dtype=np.float32)
    b_proj = np.asarray(b_proj, dtype=np.float32)

    if COMPUTE not in _NC:
        _NC[COMPUTE] = _build_nc(COMPUTE)

    in_maps = [_prep_core(c, x, W_kqv, b_kqv, padding_mask) for c in range(8)]
    res = run_bass_kernel_spmd(
        _NC[COMPUTE], in_maps, core_ids=list(range(8)), trace=TRACE, **TRACE_KW
    )
    LAST_RESULTS = res

    out = np.empty((B, T, C), np.float32)
    for b in range(B):
        out[b] = res.results[2 * b]["yout"] + res.results[2 * b + 1]["yout"]
        out[b] += b_proj[None, :]
    return out
